# revision 2
# baseline (speedup 1.0000x reference)
"""Multi-head attention (B=1, L=4096, C=512, H=8, D=64) on 8 TRN2 NeuronCores.

Sharding: head-parallel — core h computes head h end-to-end (QKV projection
for its head, softmax attention, and its partial contribution to the output
projection). Host sums the 8 partial output projections and adds the bias.

Per-core kernel layout strategy (all matmul operands pre-transposed so that
zero on-device transposes are needed):
  inputs (host-prepped):  xT [C, L] bf16, wq/wk/wv [C, D] bf16 (wq pre-scaled
                          by D^-0.5), wo [D, C] bf16
  stage 1: qT[D,L] = (wq.T @ xT-slices)  kT likewise       (PSUM->SBUF bf16)
  stage 2: v[L,D]  = (xT-tiles.T @ wv), augmented with a ones column so the
           PV matmul also accumulates the softmax row-sums ("denominator")
  attention (per i-slice of queries, j = keys):
     S.T tile [128j, iblk_i] = kT-tile.T @ qT-slice      (TensorE)
     E = exp(S.T)  PSUM->SBUF bf16                        (ScalarE, LUT exp)
     outT_aug [D+1, iblk] += v_aug-tile.T @ E             (TensorE, PSUM acc)
     row D of outT_aug = softmax denominators; bounce through DRAM to get
     them partition-major, reciprocal on VectorE
  out-proj: y-tile [128l, C] = outT-slice.T @ wo, scaled by 1/denominator
            via tensor_scalar (denominator is per-partition there)
"""

import numpy as np
import ml_dtypes

L, C, D, H = 4096, 512, 64, 8
N_CORES = 8
P = 128

_BF16 = ml_dtypes.bfloat16


def build_nc(L=L, C=C, D=D, iblk=1024, reps=1):
    import concourse.bacc as bacc
    import concourse.mybir as mybir
    import concourse.tile as tile

    f32 = mybir.dt.float32
    bf16 = mybir.dt.bfloat16
    Exp = mybir.ActivationFunctionType.Exp

    CT = C // P          # contraction tiles over channels
    LT = L // P          # key tiles (j)
    NSL = L // 512       # 512-wide l-slices for stage 1
    NI = L // iblk       # query slices
    NSUB = iblk // 512   # 512-wide matmul sub-slices per query slice
    NTI = iblk // P      # l-tiles per query slice

    nc = bacc.Bacc("TRN2", target_bir_lowering=False, debug=False)

    xt_d = nc.dram_tensor("xt", [C, L], bf16, kind="ExternalInput")
    wq_d = nc.dram_tensor("wq", [C, D], bf16, kind="ExternalInput")
    wk_d = nc.dram_tensor("wk", [C, D], bf16, kind="ExternalInput")
    wv_d = nc.dram_tensor("wv", [C, D], bf16, kind="ExternalInput")
    wo_d = nc.dram_tensor("wo", [D, C], bf16, kind="ExternalInput")
    y_d = nc.dram_tensor("y", [L, C], f32, kind="ExternalOutput")

    with tile.TileContext(nc) as tc:
        with (
            tc.tile_pool(name="const", bufs=1) as constp,
            tc.tile_pool(name="xtp", bufs=1) as xtp,
            tc.tile_pool(name="qkv", bufs=1) as qkvp,
            tc.tile_pool(name="exps", bufs=4) as expp,
            tc.tile_pool(name="aon", bufs=2) as aop,
            tc.tile_pool(name="rowp", bufs=2) as rowp,
            tc.tile_pool(name="yp", bufs=3) as yp,
            tc.tile_pool(name="drs", bufs=2, space="DRAM") as drsp,
            tc.tile_pool(name="st_ps", bufs=2, space="PSUM") as stps,
            tc.tile_pool(name="pv_ps", bufs=1, space="PSUM") as pvps,
            tc.tile_pool(name="mm_ps", bufs=2, space="PSUM") as mmps,
        ):
            # ---- load inputs to SBUF
            xt_sb = []
            for ct in range(CT):
                t = xtp.tile([P, L], bf16, name=f"xt{ct}", tag=f"xt{ct}")
                nc.sync.dma_start(t[:], xt_d[ct * P : (ct + 1) * P, :])
                xt_sb.append(t)
            wq_sb = constp.tile([P, CT, D], bf16, name="wq_sb", tag="wq")
            wk_sb = constp.tile([P, CT, D], bf16, name="wk_sb", tag="wk")
            wv_sb = constp.tile([P, CT, D], bf16, name="wv_sb", tag="wv")
            for w_sb, w_d in ((wq_sb, wq_d), (wk_sb, wk_d), (wv_sb, wv_d)):
                for ct in range(CT):
                    nc.sync.dma_start(w_sb[:, ct, :], w_d[ct * P : (ct + 1) * P, :])
            wo_sb = constp.tile([D, C], bf16, name="wo_sb", tag="wo")
            nc.sync.dma_start(wo_sb[:], wo_d[:])

            # ---- stage 1: qT, kT  [D, L] bf16
            qT = qkvp.tile([D, L], bf16, name="qT", tag="qT")
            kT = qkvp.tile([D, L], bf16, name="kT", tag="kT")
            for w_sb, dst in ((wq_sb, qT), (wk_sb, kT)):
                for ls in range(NSL):
                    ps1 = mmps.tile([P, 512], f32, name="ps1", tag="mm")
                    for ct in range(CT):
                        nc.tensor.matmul(
                            ps1[:D, :],
                            w_sb[:, ct, :],
                            xt_sb[ct][:, ls * 512 : (ls + 1) * 512],
                            start=(ct == 0),
                            stop=(ct == CT - 1),
                        )
                    nc.vector.tensor_copy(dst[:, ls * 512 : (ls + 1) * 512], ps1[:D, :])

            # ---- stage 2: v [L, D] bf16 (+ ones column for row-sums)
            v_sb = qkvp.tile([P, LT, D + 1], bf16, name="v_sb", tag="v")
            for lt in range(LT):
                ps2 = mmps.tile([P, 512], f32, name="ps2", tag="mm")
                for ct in range(CT):
                    nc.tensor.matmul(
                        ps2[:, :D],
                        xt_sb[ct][:, lt * P : (lt + 1) * P],
                        wv_sb[:, ct, :],
                        start=(ct == 0),
                        stop=(ct == CT - 1),
                    )
                nc.vector.tensor_copy(v_sb[:, lt, :D], ps2[:, :D])
            nc.vector.memset(v_sb[:, :, D], 1.0)

            # ---- attention + output projection
            for _rep in range(reps):
                for isl in range(NI):
                    pvp = pvps.tile([D + 1, iblk], f32, name="pvp", tag="pv")
                    for jt in range(LT):
                        stp = stps.tile([P, iblk], f32, name="stp", tag="st")
                        for s in range(NSUB):
                            nc.tensor.matmul(
                                stp[:, s * 512 : (s + 1) * 512],
                                kT[:, jt * P : (jt + 1) * P],
                                qT[:, isl * iblk + s * 512 : isl * iblk + (s + 1) * 512],
                                start=True,
                                stop=True,
                            )
                        e = expp.tile([P, iblk], bf16, name="e", tag="e")
                        nc.scalar.activation(e[:], stp[:], Exp)
                        for s in range(NSUB):
                            nc.tensor.matmul(
                                pvp[:, s * 512 : (s + 1) * 512],
                                v_sb[:, jt, :],
                                e[:, s * 512 : (s + 1) * 512],
                                start=(jt == 0),
                                stop=(jt == LT - 1),
                            )
                    # row D of pvp = softmax denominators for these queries.
                    # Bounce through DRAM to lay them out partition-major.
                    ao = aop.tile([D, iblk], bf16, name="ao", tag="ao")
                    nc.vector.tensor_copy(ao[:], pvp[:D, :])
                    rec_row = rowp.tile([1, iblk], f32, name="rec_row", tag="rr")
                    nc.vector.reciprocal(rec_row[:], pvp[D : D + 1, :])
                    dr = drsp.tile([iblk], f32, name="dr", tag="dr")
                    nc.sync.dma_start(dr[:], rec_row[:])
                    rec = rowp.tile([P, NTI], f32, name="rec", tag="rec")
                    nc.sync.dma_start(rec[:], dr.rearrange("(t p) -> p t", p=P))
                    for t in range(NTI):
                        pp = mmps.tile([P, 512], f32, name="pp", tag="mm")
                        for cs in range(C // 512):
                            nc.tensor.matmul(
                                pp[:, cs * 512 : (cs + 1) * 512],
                                ao[:, t * P : (t + 1) * P],
                                wo_sb[:, cs * 512 : (cs + 1) * 512],
                                start=True,
                                stop=True,
                            )
                        yt = yp.tile([P, C], f32, name="yt", tag="y")
                        nc.vector.tensor_scalar_mul(yt[:], pp[:], rec[:, t : t + 1])
                        nc.sync.dma_start(
                            y_d[isl * iblk + t * P : isl * iblk + (t + 1) * P, :], yt[:]
                        )

    nc.compile()
    return nc


_nc_cache = {}


def _get_nc(**kw):
    key = tuple(sorted(kw.items()))
    if key not in _nc_cache:
        _nc_cache[key] = build_nc(**kw)
    return _nc_cache[key]


def make_in_maps(x, w_qkv, w_out):
    """Host-side sharding: per-head weight slices, shared transposed input."""
    x = np.asarray(x, dtype=np.float32)
    w_qkv = np.asarray(w_qkv, dtype=np.float32)
    w_out = np.asarray(w_out, dtype=np.float32)
    scale = float(D) ** -0.5
    xt = np.ascontiguousarray(x[0].T).astype(_BF16)  # [C, L]
    in_maps = []
    for h in range(N_CORES):
        sl = slice(h * D, (h + 1) * D)
        wq = np.ascontiguousarray((w_qkv[0 * C :][sl, :] * scale).T).astype(_BF16)
        wk = np.ascontiguousarray(w_qkv[1 * C :][sl, :].T).astype(_BF16)
        wv = np.ascontiguousarray(w_qkv[2 * C :][sl, :].T).astype(_BF16)
        wo = np.ascontiguousarray(w_out[:, sl].T).astype(_BF16)
        in_maps.append({"xt": xt, "wq": wq, "wk": wk, "wv": wv, "wo": wo})
    return in_maps


def kernel(x, w_qkv, w_out, b_out):
    from concourse.bass_utils import run_bass_kernel_spmd

    nc = _get_nc()
    in_maps = make_in_maps(x, w_qkv, w_out)
    res = run_bass_kernel_spmd(nc, in_maps, list(range(N_CORES)))
    y = res.results[0]["y"].astype(np.float64)
    for i in range(1, N_CORES):
        y += res.results[i]["y"]
    y = (y + np.asarray(b_out, dtype=np.float64)).astype(np.float32)
    return y[None]
